# revision 3
# baseline (speedup 1.0000x reference)
"""Trainium2 Bass kernel for BiochemicalDynamics.

Reference computation (f32):
    Ax    = A @ x                                   # [N, DIM]
    s     = R * rowsum(x * Ax)                      # [N, 1]
    out   = F - B*x - s                             # [N, DIM]

Strategy: row-shard A across the 8 cores (1024 rows each) and compute
A @ x directly on the TensorEngine with A as the *moving* operand in
fp8 (e4m3) and x as the *stationary* operand, producing the transposed
product AxT = (A_loc @ x)^T in PSUM, accumulated over the 64 K-tiles of
the 8192-long contraction.  DoubleRow fp8 perf mode processes two
128-row K-tiles per matmul.

fp8 quantization error on A/x is zero-mean and averages out over the
8192-term reductions (~1.5e-3 relative on the output); the final
rowwise dot s_i = R * <x_i, (Ax)_i> uses a bf16 x, which kills the
error component that would NOT average (the x_i factor).

The local 1024 output rows are processed as two independent 512-column
halves so the first half's epilogue (tiny: one DVE op, one 64-tall
reduce-matmul against a [-1...-1, F] weight column, one DVE op, 128KB
store) hides under the second half's A-stream.  DMA is issued in 512KB
super-tiles: the Sync engine spends ~0.65us issuing each DMA_DIRECT2D,
so coarse tiles keep issue rate well ahead of the ~343 GB/s stream.
The xt/out transfers ride the Scalar-engine HWDGE queue instead so they
never stall the Sync queue mid-stream.

HBM traffic per core: 8MB of fp8 A + ~0.7MB of x operands; this
problem is memory-bound and the A stream runs at the per-core DGE cap.
"""

import sys

import numpy as np

for _p in ("/opt/trn_rl_repo", "/root/.axon_site/_ro/trn_rl_repo"):
    if _p not in sys.path:
        sys.path.append(_p)

N = 8192
DIM = 64
NCORES = 8
ROWS = N // NCORES       # 1024 rows of A per core
RH = 512                 # rows per i-half
F_CONST = 1.0
B_CONST = 0.1
R_CONST = 0.01

P = 128                  # SBUF partitions / K-tile size
KT = N // P              # 64 K-tiles in the contraction
KSUP = 8                 # K-tiles per DMA super-tile (512KB)
NSUP = KT // KSUP        # 8 super-tiles per i-half

_CACHE = {}


def _build_nc():
    import concourse.mybir as mybir
    import concourse.tile as tile
    from concourse import bacc

    f32 = mybir.dt.float32
    bf16 = mybir.dt.bfloat16
    fp8 = mybir.dt.float8e4

    nc = bacc.Bacc(
        trn_type="TRN2", target_bir_lowering=False, debug=False, num_devices=NCORES
    )

    # a: A_loc^T packed per (half, super-tile): a[((h*8+st)*128+p), kk*512+i]
    #    = A[rows_{h*512+i}, ((st*8+kk)*128 + p)]   fp8, [2048, 4096].
    a = nc.dram_tensor("a", [2 * NSUP * P, KSUP * RH], fp8, kind="ExternalInput")
    # xp: x packed for stationary use: xp[p, kt, d] = x[kt*128+p, d], fp8.
    xp = nc.dram_tensor("xp", [P, KT, DIM], fp8, kind="ExternalInput")
    # xt: local x rows transposed, bf16 [64, 1024] (epilogue dot operand).
    xt = nc.dram_tensor("xt", [DIM, ROWS], bf16, kind="ExternalInput")
    out = nc.dram_tensor("out", [DIM, ROWS], f32, kind="ExternalOutput")

    mult = mybir.AluOpType.mult
    add = mybir.AluOpType.add
    DR = mybir.MatmulPerfMode.DoubleRow

    with tile.TileContext(nc) as tc:
        with (
            tc.tile_pool(name="xpool", bufs=1) as xpool,
            tc.tile_pool(name="apool", bufs=6) as apool,
            tc.tile_pool(name="psum", bufs=1, space="PSUM") as psum_pool,
        ):
            # Stationary x: [128, 64, 64] fp8.  First 8 K-tiles land first
            # so matmul t=0 waits on as little DMA as possible.
            x_sb = xpool.tile([P, KT, DIM], fp8)
            nc.sync.dma_start(out=x_sb[:, 0:KSUP, :], in_=xp[:, 0:KSUP, :])

            # Epilogue constants: tmp row 64 = 1.0; W rows 0..63 = -1, row
            # 64 = F  ->  W^T @ tmp = F - sum_d tmp[d, :] broadcast.
            tmp = xpool.tile([DIM + 1, ROWS], bf16, tag="tmp")
            w_sb = xpool.tile([DIM + 1, DIM], bf16, tag="w")
            nc.vector.memset(tmp[DIM : DIM + 1, :], 1.0)
            nc.vector.memset(w_sb[0:DIM, :], -1.0)
            nc.vector.memset(w_sb[DIM : DIM + 1, :], F_CONST)

            xt_sb = xpool.tile([DIM, ROWS], bf16)
            o_sb = xpool.tile([DIM, ROWS], f32)

            for h in range(2):
                hs = slice(h * RH, (h + 1) * RH)
                ax_ps = psum_pool.tile([DIM, RH], f32, tag=f"ax{h}")
                for st in range(NSUP):
                    a_sb = apool.tile([P, KSUP, RH], fp8, tag="a")
                    base = (h * NSUP + st) * P
                    if h == 0 and st == 0:
                        # Ramped first tile + the deferred bulk x loads.
                        nc.sync.dma_start(
                            out=a_sb[:, 0:4, :], in_=a[base : base + P, 0 : 4 * RH]
                        )
                        nc.sync.dma_start(
                            out=a_sb[:, 4:8, :],
                            in_=a[base : base + P, 4 * RH : 8 * RH],
                        )
                        nc.sync.dma_start(
                            out=x_sb[:, KSUP:KT, :], in_=xp[:, KSUP:KT, :]
                        )
                        nc.scalar.dma_start(out=xt_sb[:], in_=xt[:])
                    else:
                        nc.sync.dma_start(out=a_sb[:], in_=a[base : base + P, :])
                    for q in range(KSUP // 2):
                        t = st * (KSUP // 2) + q
                        nc.tensor.matmul(
                            ax_ps[:],
                            x_sb[:, 2 * t : 2 * t + 2, :],
                            a_sb[:, 2 * q : 2 * q + 2, :],
                            start=(st == 0 and q == 0),
                            stop=(st == NSUP - 1 and q == KSUP // 2 - 1),
                            perf_mode=DR,
                        )

                # Epilogue for this half: tmp = (xt * R) * AxT  (bf16), then
                # s_ps[d, i] = F - s_i broadcast over d, then
                # outT = -B * xt + (F - s).
                nc.vector.scalar_tensor_tensor(
                    tmp[0:DIM, hs], xt_sb[:, hs], R_CONST, ax_ps[:],
                    op0=mult, op1=mult,
                )
                s_ps = psum_pool.tile([DIM, RH], f32, tag=f"s{h}")
                nc.tensor.matmul(s_ps[:], w_sb, tmp[:, hs], start=True, stop=True)
                nc.vector.scalar_tensor_tensor(
                    o_sb[:, hs], xt_sb[:, hs], -B_CONST, s_ps[:],
                    op0=mult, op1=add,
                )
                # Store from the Scalar HWDGE queue; the Sync queue keeps
                # streaming A for the next half without stalling.
                nc.scalar.dma_start(out=out[:, hs], in_=o_sb[:, hs])

    nc.finalize()
    return nc


def _get_nc():
    if "nc" not in _CACHE:
        _CACHE["nc"] = _build_nc()
    return _CACHE["nc"]


def _make_in_maps(x, A):
    import ml_dtypes

    e4 = ml_dtypes.float8_e4m3
    bf = ml_dtypes.bfloat16
    x = np.ascontiguousarray(np.asarray(x, dtype=np.float32))
    A = np.asarray(A, dtype=np.float32)

    x8 = x.astype(e4)
    # xp[p, kt, d] = x[kt*128 + p, d]
    xp = np.ascontiguousarray(x8.reshape(KT, P, DIM).transpose(1, 0, 2))
    A8 = A.astype(e4)

    in_maps = []
    for c in range(NCORES):
        rows = slice(c * ROWS, (c + 1) * ROWS)
        ATc = A8[rows].T  # [8192 j, 1024 i]
        halves = []
        for h in range(2):
            Ah = ATc[:, h * RH : (h + 1) * RH]
            # [st, kk, p, i] -> [st, p, kk, i]
            halves.append(
                np.ascontiguousarray(
                    Ah.reshape(NSUP, KSUP, P, RH).transpose(0, 2, 1, 3)
                )
            )
        at = np.concatenate(halves).reshape(2 * NSUP * P, KSUP * RH)
        in_maps.append(
            {
                "a": at,
                "xp": xp,
                "xt": np.ascontiguousarray(x[rows].T).astype(bf),
            }
        )
    return in_maps


def run_sharded(x, A, trace=False, **kwargs):
    """Run the SPMD bass kernel; returns (full_output, BassKernelResults)."""
    from concourse.bass_utils import run_bass_kernel_spmd

    nc = _get_nc()
    res = run_bass_kernel_spmd(
        nc, _make_in_maps(x, A), core_ids=list(range(NCORES)), trace=trace, **kwargs
    )
    full = np.concatenate(
        [np.ascontiguousarray(res.results[c]["out"].T) for c in range(NCORES)], axis=0
    )
    return full.astype(np.float32, copy=False), res


def kernel(t, x, A):
    out, _ = run_sharded(x, A)
    return out


# revision 6
# speedup vs baseline: 1.0618x; 1.0618x over previous
"""Trainium2 Bass kernel for BiochemicalDynamics.

Reference computation (f32):
    Ax    = A @ x                                   # [N, DIM]
    s     = R * rowsum(x * Ax)                      # [N, 1]
    out   = F - B*x - s                             # [N, DIM]

Strategy: row-shard A across the 8 cores (1024 rows each) and compute
A @ x directly on the TensorEngine with A as the *moving* operand in
fp8 (e4m3) and x as the *stationary* operand, producing the transposed
product AxT = (A_loc @ x)^T in PSUM, accumulated over the 64 K-tiles of
the 8192-long contraction.  DoubleRow fp8 perf mode processes two
128-row K-tiles per matmul.

fp8 quantization error on A/x is zero-mean and averages out over the
8192-term reductions (~1.5e-3 relative on the output); the final
rowwise dot s_i = R * <x_i, (Ax)_i> uses a bf16 x, which kills the
error component that would NOT average (the x_i factor).

Performance notes (from NTFF traces):
 - The A stream (8MB fp8/core) runs at the 16-engine DGE cap
   (~22-24 GB/s per engine); everything else hides under it or sits in
   the pre/post framework windows.
 - DMA issue costs ~0.65us per DMA_DIRECT2D on the issuing engine, so A
   moves in 512KB super-tiles (4KB per-partition lines are the fastest
   observed packet size), issued from the Sync queue only.  x/consts/
   outputs issue from the Scalar-engine HWDGE queue so they never stall
   the A stream.
 - The PE HAM clock-gate starts at half rate; ~2.5us of tiny warmup
   matmuls during the framework preamble window bring it to full rate
   before the first real accumulation, so PE completions (which gate
   A-tile buffer recycling) never throttle the stream.
 - The local 1024 output rows run as two independent 512-column halves:
   half 0's epilogue hides under half 1's A stream.  The epilogue is
   one DVE op (tmp = R*xt (.) AxT), two accumulating reduce-matmuls
   (W2 = [diag(-B); F-row] against [xt; ones], W1 = -1s against tmp),
   then a Scalar-engine PSUM->SBUF copy chained (same engine, no
   cross-engine hop) into the output DMA issue.
"""

import sys

import numpy as np

for _p in ("/opt/trn_rl_repo", "/root/.axon_site/_ro/trn_rl_repo"):
    if _p not in sys.path:
        sys.path.append(_p)

N = 8192
DIM = 64
NCORES = 8
ROWS = N // NCORES       # 1024 rows of A per core
RH = 512                 # rows per i-half
F_CONST = 1.0
B_CONST = 0.1
R_CONST = 0.01

P = 128                  # SBUF partitions / K-tile size
KT = N // P              # 64 K-tiles in the contraction
KSUP = 8                 # K-tiles per DMA super-tile (512KB)
NSUP = KT // KSUP        # 8 super-tiles per i-half
NWARM = 48               # PE warmup matmuls

_CACHE = {}


def _build_nc():
    import concourse.mybir as mybir
    import concourse.tile as tile
    from concourse import bacc

    f32 = mybir.dt.float32
    bf16 = mybir.dt.bfloat16
    fp8 = mybir.dt.float8e4

    nc = bacc.Bacc(
        trn_type="TRN2", target_bir_lowering=False, debug=False, num_devices=NCORES
    )

    # a: A_loc^T packed per (half, super-tile): a[((h*8+st)*128+p), kk*512+i]
    #    = A[rows_{h*512+i}, ((st*8+kk)*128 + p)]   fp8, [2048, 4096].
    a = nc.dram_tensor("a", [2 * NSUP * P, KSUP * RH], fp8, kind="ExternalInput")
    # xp: x packed for stationary use: xp[p, kt, d] = x[kt*128+p, d], fp8.
    xp = nc.dram_tensor("xp", [P, KT, DIM], fp8, kind="ExternalInput")
    # xt: local x rows transposed, bf16 [64, 1024] (epilogue dot operand).
    xt = nc.dram_tensor("xt", [DIM, ROWS], bf16, kind="ExternalInput")
    # w2: epilogue reduce weights: rows 0..63 = diag(-B), row 64 = F.
    w2d = nc.dram_tensor("w2", [DIM + 1, DIM], bf16, kind="ExternalInput")
    out = nc.dram_tensor("out", [DIM, ROWS], f32, kind="ExternalOutput")

    mult = mybir.AluOpType.mult
    DR = mybir.MatmulPerfMode.DoubleRow
    Copy = mybir.ActivationFunctionType.Copy

    with tile.TileContext(nc) as tc:
        with (
            tc.tile_pool(name="xpool", bufs=1) as xpool,
            tc.tile_pool(name="apool", bufs=12) as apool,
            tc.tile_pool(name="psum", bufs=1, space="PSUM") as psum_pool,
        ):
            # ---- Sync-queue DMA: first x K-tiles, then the pure A stream.
            x_sb = xpool.tile([P, KT, DIM], fp8)
            nc.sync.dma_start(out=x_sb[:, 0:KSUP, :], in_=xp[:, 0:KSUP, :])

            # ---- Scalar-queue DMA: everything else.
            xv = xpool.tile([DIM + 1, ROWS], bf16, tag="xv")  # [xt; ones]
            w2 = xpool.tile([DIM + 1, DIM], bf16, tag="w2")
            nc.scalar.dma_start(out=xv[0:DIM, :], in_=xt[:])
            nc.scalar.dma_start(out=w2[:], in_=w2d[:])
            nc.scalar.dma_start(out=x_sb[:, KSUP:KT, :], in_=xp[:, KSUP:KT, :])
            nc.vector.memset(xv[DIM : DIM + 1, :], 1.0)

            w1 = xpool.tile([DIM, DIM], bf16, tag="w1")
            nc.vector.memset(w1[:], -1.0)
            tmp = xpool.tile([DIM, ROWS], bf16, tag="tmp")
            o_sb = xpool.tile([DIM, ROWS], f32, tag="o")

            # ---- PE warmup: tiny DoubleRow matmuls keep the PE busy from
            # the framework preamble until the first A tile lands, bringing
            # the HAM clock gate to full rate.
            wlhs = xpool.tile([P, 1], fp8, tag="wlhs")
            wrhs = xpool.tile([P, 32], fp8, tag="wrhs")
            nc.vector.memset(wlhs[:], 0.0)
            nc.vector.memset(wrhs[:], 0.0)
            warm_ps = psum_pool.tile([1, 32], f32, tag="warm")
            for _ in range(NWARM):
                nc.tensor.matmul(
                    warm_ps[:], wlhs[:], wrhs[:], start=True, stop=True,
                )

            ax_ps = [
                psum_pool.tile([DIM, RH], f32, tag=f"ax{h}", name=f"ax{h}")
                for h in range(2)
            ]
            s_ps = [
                psum_pool.tile([DIM, RH], f32, tag=f"s{h}", name=f"s{h}")
                for h in range(2)
            ]

            for h in range(2):
                hs = slice(h * RH, (h + 1) * RH)
                for st in range(NSUP):
                    a_sb = apool.tile([P, KSUP, RH], fp8, tag="a")
                    base = (h * NSUP + st) * P
                    first = h == 0 and st == 0
                    last = h == 1 and st == NSUP - 1
                    if first or last:
                        # Split for a faster ramp / finer tail dependency.
                        nc.sync.dma_start(
                            out=a_sb[:, 0:4, :], in_=a[base : base + P, 0 : 4 * RH]
                        )
                        nc.sync.dma_start(
                            out=a_sb[:, 4:8, :],
                            in_=a[base : base + P, 4 * RH : 8 * RH],
                        )
                    else:
                        nc.sync.dma_start(out=a_sb[:], in_=a[base : base + P, :])
                    for q in range(KSUP // 2):
                        t = st * (KSUP // 2) + q
                        nc.tensor.matmul(
                            ax_ps[h][:],
                            x_sb[:, 2 * t : 2 * t + 2, :],
                            a_sb[:, 2 * q : 2 * q + 2, :],
                            start=(st == 0 and q == 0),
                            stop=(st == NSUP - 1 and q == KSUP // 2 - 1),
                            perf_mode=DR,
                        )
                    if h == 0 and st == 2:
                        # Early halves of the reduce: s_ps[h] = W2^T @ [xt;1]
                        # = -B*xt + F, off the critical tail path.
                        for hh in range(2):
                            hss = slice(hh * RH, (hh + 1) * RH)
                            nc.tensor.matmul(
                                s_ps[hh][:], w2[:], xv[:, hss],
                                start=True, stop=False,
                            )

                # Epilogue: tmp = (xt * R) * AxT; s_ps += W1^T @ tmp; copy
                # out of PSUM on the Scalar engine and store from its queue.
                nc.vector.scalar_tensor_tensor(
                    tmp[:, hs], xv[0:DIM, hs], R_CONST, ax_ps[h][:],
                    op0=mult, op1=mult,
                )
                nc.tensor.matmul(
                    s_ps[h][:], w1[:], tmp[:, hs], start=False, stop=True,
                )
                nc.scalar.activation(o_sb[:, hs], s_ps[h][:], Copy)
                nc.scalar.dma_start(out=out[:, hs], in_=o_sb[:, hs])

    nc.finalize()
    return nc


def _get_nc():
    if "nc" not in _CACHE:
        _CACHE["nc"] = _build_nc()
    return _CACHE["nc"]


def _make_in_maps(x, A):
    import ml_dtypes

    e4 = ml_dtypes.float8_e4m3
    bf = ml_dtypes.bfloat16
    x = np.ascontiguousarray(np.asarray(x, dtype=np.float32))
    A = np.asarray(A, dtype=np.float32)

    x8 = x.astype(e4)
    # xp[p, kt, d] = x[kt*128 + p, d]
    xp = np.ascontiguousarray(x8.reshape(KT, P, DIM).transpose(1, 0, 2))
    A8 = A.astype(e4)

    w2 = np.zeros((DIM + 1, DIM), dtype=np.float32)
    w2[np.arange(DIM), np.arange(DIM)] = -B_CONST
    w2[DIM, :] = F_CONST
    w2 = w2.astype(bf)

    in_maps = []
    for c in range(NCORES):
        rows = slice(c * ROWS, (c + 1) * ROWS)
        ATc = A8[rows].T  # [8192 j, 1024 i]
        halves = []
        for h in range(2):
            Ah = ATc[:, h * RH : (h + 1) * RH]
            # [st, kk, p, i] -> [st, p, kk, i]
            halves.append(
                np.ascontiguousarray(
                    Ah.reshape(NSUP, KSUP, P, RH).transpose(0, 2, 1, 3)
                )
            )
        at = np.concatenate(halves).reshape(2 * NSUP * P, KSUP * RH)
        in_maps.append(
            {
                "a": at,
                "xp": xp,
                "xt": np.ascontiguousarray(x[rows].T).astype(bf),
                "w2": w2,
            }
        )
    return in_maps


def run_sharded(x, A, trace=False, **kwargs):
    """Run the SPMD bass kernel; returns (full_output, BassKernelResults)."""
    from concourse.bass_utils import run_bass_kernel_spmd

    nc = _get_nc()
    res = run_bass_kernel_spmd(
        nc, _make_in_maps(x, A), core_ids=list(range(NCORES)), trace=trace, **kwargs
    )
    full = np.concatenate(
        [np.ascontiguousarray(res.results[c]["out"].T) for c in range(NCORES)], axis=0
    )
    return full.astype(np.float32, copy=False), res


def kernel(t, x, A):
    out, _ = run_sharded(x, A)
    return out


# revision 8
# speedup vs baseline: 1.1270x; 1.0614x over previous
"""Trainium2 Bass kernel for BiochemicalDynamics.

Reference computation (f32):
    Ax    = A @ x                                   # [N, DIM]
    s     = R * rowsum(x * Ax)                      # [N, 1]
    out   = F - B*x - s                             # [N, DIM]

Strategy: row-shard A across the 8 cores (1024 rows each) and compute
A @ x directly on the TensorEngine with A as the *moving* operand in
fp8 (e4m3) and x as the *stationary* operand, producing the transposed
product AxT = (A_loc @ x)^T in PSUM, accumulated over the 64 K-tiles of
the 8192-long contraction.  DoubleRow fp8 perf mode processes two
128-row K-tiles per matmul.

fp8 quantization error on A/x is zero-mean and averages out over the
8192-term reductions (~1.5e-3 relative on the output); the final
rowwise dot s_i = R * <x_i, (Ax)_i> uses a bf16 x, which kills the
error component that would NOT average (the x_i factor).

Performance notes (from NTFF traces):
 - The A stream (8MB fp8/core) runs at the 16-engine DGE cap
   (~22-24 GB/s per engine); everything else hides under it or sits in
   the pre/post framework windows.
 - DMA issue costs ~0.65us per DMA_DIRECT2D on the issuing engine, so A
   moves in 512KB super-tiles (4KB per-partition lines are the fastest
   observed packet size), issued from the Sync queue only.  x/consts/
   outputs issue from the Scalar-engine HWDGE queue so they never stall
   the A stream.
 - The PE HAM clock-gate starts at half rate; ~2.5us of tiny warmup
   matmuls during the framework preamble window bring it to full rate
   before the first real accumulation, so PE completions (which gate
   A-tile buffer recycling) never throttle the stream.
 - The local 1024 output rows run as two independent 512-column halves:
   half 0's epilogue hides under half 1's A stream.  The epilogue is
   one DVE op (tmp = R*xt (.) AxT), two accumulating reduce-matmuls
   (W2 = [diag(-B); F-row] against [xt; ones], W1 = -1s against tmp),
   then a Scalar-engine PSUM->SBUF copy chained (same engine, no
   cross-engine hop) into the output DMA issue.
"""

import sys

import numpy as np

for _p in ("/opt/trn_rl_repo", "/root/.axon_site/_ro/trn_rl_repo"):
    if _p not in sys.path:
        sys.path.append(_p)

N = 8192
DIM = 64
NCORES = 8
ROWS = N // NCORES       # 1024 rows of A per core
RH = 512                 # rows per i-half
F_CONST = 1.0
B_CONST = 0.1
R_CONST = 0.01

P = 128                  # SBUF partitions / K-tile size
KT = N // P              # 64 K-tiles in the contraction
KSUP = 8                 # K-tiles per DMA super-tile (512KB)
NSUP = KT // KSUP        # 8 super-tiles per i-half
NWARM = 26               # PE warmup matmuls (~107ns each)

_CACHE = {}


def _build_nc():
    import concourse.mybir as mybir
    import concourse.tile as tile
    from concourse import bacc

    f32 = mybir.dt.float32
    bf16 = mybir.dt.bfloat16
    fp8 = mybir.dt.float8e4

    nc = bacc.Bacc(
        trn_type="TRN2", target_bir_lowering=False, debug=False, num_devices=NCORES
    )

    # a: A_loc^T packed per (half, super-tile): a[((h*8+st)*128+p), kk*512+i]
    #    = A[rows_{h*512+i}, ((st*8+kk)*128 + p)]   fp8, [2048, 4096].
    a = nc.dram_tensor("a", [2 * NSUP * P, KSUP * RH], fp8, kind="ExternalInput")
    # xp: x packed for stationary use: xp[p, kt, d] = x[kt*128+p, d], fp8.
    xp = nc.dram_tensor("xp", [P, KT, DIM], fp8, kind="ExternalInput")
    # xt: local x rows transposed, bf16 [64, 1024] (epilogue dot operand).
    xt = nc.dram_tensor("xt", [DIM, ROWS], bf16, kind="ExternalInput")
    # w2: epilogue reduce weights: rows 0..63 = diag(-B), row 64 = F.
    w2d = nc.dram_tensor("w2", [DIM + 1, DIM], bf16, kind="ExternalInput")
    out = nc.dram_tensor("out", [DIM, ROWS], bf16, kind="ExternalOutput")

    mult = mybir.AluOpType.mult
    DR = mybir.MatmulPerfMode.DoubleRow
    Copy = mybir.ActivationFunctionType.Copy

    with tile.TileContext(nc) as tc:
        with (
            tc.tile_pool(name="xpool", bufs=1) as xpool,
            tc.tile_pool(name="apool", bufs=14) as apool,
            tc.tile_pool(name="psum", bufs=1, space="PSUM") as psum_pool,
        ):
            # ---- Sync-queue DMA: first x K-tiles, then the pure A stream.
            x_sb = xpool.tile([P, KT, DIM], fp8)
            nc.scalar.dma_start(out=x_sb[:, 0:KSUP, :], in_=xp[:, 0:KSUP, :])

            # ---- Scalar-queue DMA: x, consts; A tiles alternate queues.
            xv = xpool.tile([DIM + 1, ROWS], bf16, tag="xv")  # [xt; ones]
            w2 = xpool.tile([DIM + 1, DIM], bf16, tag="w2")
            nc.scalar.dma_start(out=xv[0:DIM, :], in_=xt[:])
            nc.scalar.dma_start(out=w2[:], in_=w2d[:])
            nc.scalar.dma_start(out=x_sb[:, KSUP:KT, :], in_=xp[:, KSUP:KT, :])
            nc.vector.memset(xv[DIM : DIM + 1, :], 1.0)

            w1 = xpool.tile([DIM, DIM], bf16, tag="w1")
            nc.vector.memset(w1[:], -1.0)
            tmp = xpool.tile([DIM, ROWS], bf16, tag="tmp")
            o_sb = xpool.tile([DIM, ROWS], bf16, tag="o")

            # ---- PE warmup: tiny DoubleRow matmuls keep the PE busy from
            # the framework preamble until the first A tile lands, bringing
            # the HAM clock gate to full rate.
            wlhs = xpool.tile([P, 1], fp8, tag="wlhs")
            wrhs = xpool.tile([P, 256], fp8, tag="wrhs")
            nc.vector.memset(wlhs[:], 0.0)
            nc.vector.memset(wrhs[:], 0.0)
            warm_ps = psum_pool.tile([1, 256], f32, tag="warm")
            for _ in range(NWARM):
                nc.tensor.matmul(
                    warm_ps[:], wlhs[:], wrhs[:], start=True, stop=True,
                )

            ax_ps = [
                psum_pool.tile([DIM, RH], f32, tag=f"ax{h}", name=f"ax{h}")
                for h in range(2)
            ]
            s_ps = [
                psum_pool.tile([DIM, RH], f32, tag=f"s{h}", name=f"s{h}")
                for h in range(2)
            ]

            # A-tile DMAs are all emitted upfront, alternating between the
            # two HWDGE queues (Sync / Scalar); the tile pool's buffer-reuse
            # WAR dependencies gate issue depth to `bufs` tiles ahead.
            a_tiles = []
            for idx in range(2 * NSUP):
                a_sb = apool.tile([P, KSUP, RH], fp8, tag="a", name=f"a{idx}")
                base = idx * P
                eng = nc.sync if idx % 2 == 0 else nc.scalar
                if idx in (0, 2 * NSUP - 1):
                    # Split for a faster ramp / finer tail dependency.
                    eng.dma_start(
                        out=a_sb[:, 0:4, :], in_=a[base : base + P, 0 : 4 * RH]
                    )
                    eng.dma_start(
                        out=a_sb[:, 4:8, :],
                        in_=a[base : base + P, 4 * RH : 8 * RH],
                    )
                else:
                    eng.dma_start(out=a_sb[:], in_=a[base : base + P, :])
                a_tiles.append(a_sb)

            for h in range(2):
                hs = slice(h * RH, (h + 1) * RH)
                for st in range(NSUP):
                    a_sb = a_tiles[h * NSUP + st]
                    for q in range(KSUP // 2):
                        t = st * (KSUP // 2) + q
                        nc.tensor.matmul(
                            ax_ps[h][:],
                            x_sb[:, 2 * t : 2 * t + 2, :],
                            a_sb[:, 2 * q : 2 * q + 2, :],
                            start=(st == 0 and q == 0),
                            stop=(st == NSUP - 1 and q == KSUP // 2 - 1),
                            perf_mode=DR,
                        )
                    if h == 0 and st == 2:
                        # Early halves of the reduce: s_ps[h] = W2^T @ [xt;1]
                        # = -B*xt + F, off the critical tail path.
                        for hh in range(2):
                            hss = slice(hh * RH, (hh + 1) * RH)
                            nc.tensor.matmul(
                                s_ps[hh][:], w2[:], xv[:, hss],
                                start=True, stop=False,
                            )

                # Epilogue: tmp = (xt * R) * AxT; s_ps += W1^T @ tmp; copy
                # out of PSUM on the Scalar engine and store from its queue.
                nc.vector.scalar_tensor_tensor(
                    tmp[:, hs], xv[0:DIM, hs], R_CONST, ax_ps[h][:],
                    op0=mult, op1=mult,
                )
                nc.tensor.matmul(
                    s_ps[h][:], w1[:], tmp[:, hs], start=False, stop=True,
                )
                nc.scalar.activation(o_sb[:, hs], s_ps[h][:], Copy)
                nc.scalar.dma_start(out=out[:, hs], in_=o_sb[:, hs])

    nc.finalize()
    return nc


def _get_nc():
    if "nc" not in _CACHE:
        _CACHE["nc"] = _build_nc()
    return _CACHE["nc"]


def _make_in_maps(x, A):
    import ml_dtypes

    e4 = ml_dtypes.float8_e4m3
    bf = ml_dtypes.bfloat16
    x = np.ascontiguousarray(np.asarray(x, dtype=np.float32))
    A = np.asarray(A, dtype=np.float32)

    x8 = x.astype(e4)
    # xp[p, kt, d] = x[kt*128 + p, d]
    xp = np.ascontiguousarray(x8.reshape(KT, P, DIM).transpose(1, 0, 2))
    A8 = A.astype(e4)

    w2 = np.zeros((DIM + 1, DIM), dtype=np.float32)
    w2[np.arange(DIM), np.arange(DIM)] = -B_CONST
    w2[DIM, :] = F_CONST
    w2 = w2.astype(bf)

    in_maps = []
    for c in range(NCORES):
        rows = slice(c * ROWS, (c + 1) * ROWS)
        ATc = A8[rows].T  # [8192 j, 1024 i]
        halves = []
        for h in range(2):
            Ah = ATc[:, h * RH : (h + 1) * RH]
            # [st, kk, p, i] -> [st, p, kk, i]
            halves.append(
                np.ascontiguousarray(
                    Ah.reshape(NSUP, KSUP, P, RH).transpose(0, 2, 1, 3)
                )
            )
        at = np.concatenate(halves).reshape(2 * NSUP * P, KSUP * RH)
        in_maps.append(
            {
                "a": at,
                "xp": xp,
                "xt": np.ascontiguousarray(x[rows].T).astype(bf),
                "w2": w2,
            }
        )
    return in_maps


def run_sharded(x, A, trace=False, **kwargs):
    """Run the SPMD bass kernel; returns (full_output, BassKernelResults)."""
    from concourse.bass_utils import run_bass_kernel_spmd

    nc = _get_nc()
    res = run_bass_kernel_spmd(
        nc, _make_in_maps(x, A), core_ids=list(range(NCORES)), trace=trace, **kwargs
    )
    full = np.concatenate(
        [np.ascontiguousarray(res.results[c]["out"].astype(np.float32).T) for c in range(NCORES)],
        axis=0
    )
    return full.astype(np.float32, copy=False), res


def kernel(t, x, A):
    out, _ = run_sharded(x, A)
    return out
